# revision 1
# baseline (speedup 1.0000x reference)
"""Trainium2 Bass kernel for nn_VELORA_34488587387269 (moe_routing).

Strategy (see spec sharding_hint): data-parallel over the batch — each of the
8 NeuronCores gets B/8 = 1024 samples; all weights are replicated.  The whole
computation runs in transposed layout (x^T: [D, B_loc]) so every stored weight
matrix [in, out] serves directly as the matmul stationary operand (lhsT).

Precision: the expert/fusion matmuls run in float32r (full PE rate, ~1.2e-4
operand rounding).  The router trunk (h and the domain logits that pick the
expert) runs in true fp32 — an argmax flip on the expert selector changes a
whole row of the output, so that chain cannot tolerate reduced precision.

Per-sample routing (argmax one-hots, softmax weight of the chosen expert) is
computed in [B, heads] orientation on 128 lanes, PE-transposed back to
[heads, B], and the data-dependent embedding gather becomes a one-hot matmul.
"""

import sys
import numpy as np

sys.path.insert(0, "/opt/trn_rl_repo")

import concourse.bass as bass  # noqa: E402
import concourse.tile as tile  # noqa: E402
import concourse.mybir as mybir  # noqa: E402
from concourse import bacc  # noqa: E402
from concourse.masks import make_identity  # noqa: E402
from concourse.bass_utils import run_bass_kernel_spmd  # noqa: E402

P = 128
B, D, HR, HE, HF = 8192, 1024, 512, 4096, 2048
N_OPS, N_TASKS = 4, 4
NCORES = 8
B_LOC = B // NCORES          # 1024 samples per core
NB = B_LOC // P              # 8 batch chunks
KD = D // P                  # 8 contraction chunks over D
NHR = HR // P                # 4 router hidden chunks
NQ = 4                       # HE quarters
HEQ = HE // NQ               # 1024
NHEQ = HEQ // P              # 8
NF = 2                       # HF halves
HFQ = HF // NF               # 1024
NHFQ = HFQ // P              # 8
NH = B_LOC // 512            # matmul free-dim halves (N=512 each)

f32 = mybir.dt.float32
f32r = mybir.dt.float32r
AF = mybir.ActivationFunctionType
ALU = mybir.AluOpType

last_exec_time_ns = None


def _emit(nc, tc, ctx, dram):
    (xT_d, r1h_d, wcat_d, w1h_d, w2h_d, l1h_d, l2h_d, f1h_d, f2h_d,
     opemb_d, taskemb_d, br1_d, bm1_d, bm2_d, bl1_d, bl2_d, bf1_d, bf2_d,
     cdom_d, outT_d) = dram

    # ---- pools ----
    big = ctx.enter_context(tc.tile_pool(name="big", bufs=NB))
    wts = ctx.enter_context(tc.tile_pool(name="wts", bufs=5))
    wr = ctx.enter_context(tc.tile_pool(name="wr", bufs=4))
    hp = ctx.enter_context(tc.tile_pool(name="hp", bufs=4))
    sm = ctx.enter_context(tc.tile_pool(name="sm", bufs=1))
    tmp = ctx.enter_context(tc.tile_pool(name="tmp", bufs=1))
    pp = ctx.enter_context(tc.tile_pool(name="pp", bufs=2, space="PSUM"))

    def dma(out_ap, in_ap):
        nc.sync.dma_start(out=out_ap, in_=in_ap)

    # ---- first router weight tiles go ahead of everything (64KB each) ----
    wr_pre = []
    for k in range(4):
        wt = wr.tile([P, P], f32, tag="wr1", name="wr1")
        dma(wt[:], r1h_d[k, :, 0:P])
        wr_pre.append(wt)

    # ---- load x^T: two half-DMAs per chunk to spread across queues ----
    xk = []
    for k in range(KD):
        t = big.tile([P, B_LOC], f32, tag="xin", name="xin")
        dma(t[:, 0:512], xT_d[k * P:(k + 1) * P, 0:512])
        dma(t[:, 512:1024], xT_d[k * P:(k + 1) * P, 512:1024])
        xk.append(t)

    # early-needed small tensors
    br1 = sm.tile([P, NHR], f32, tag="br1", name="br1")
    dma(br1[:], br1_d[:])
    wcat = sm.tile([P, 4 * 10], f32, tag="wcat", name="wcat")
    dma(wcat[:], wcat_d[:])
    cdom = sm.tile([P, 1], f32, tag="cdom", name="cdom")
    dma(cdom[:], cdom_d[:])
    ident = sm.tile([P, P], f32, tag="ident", name="ident")
    make_identity(nc, ident[:])
    sigwarm = sm.tile([P, 1], f32, tag="sigwarm", name="sigwarm")
    nc.vector.memset(sigwarm[:], 0.0)
    nc.scalar.activation(sigwarm[:], sigwarm[:], AF.Sigmoid)
    ones1f = sm.tile([1, P], f32, tag="ones1f", name="ones1f")
    nc.vector.memset(ones1f[:], 1.0)
    ones1 = sm.tile([1, P], f32r, tag="ones1", name="ones1")
    nc.vector.tensor_copy(ones1[:], ones1f[:])

    # ---- router: h^T = relu(W_r1^T @ x^T + b_r1), fp32 ----
    hrt = []
    for m in range(NHR):
        ps = pp.tile([P, B_LOC], f32, tag="pA", name="ph")
        for k in range(KD):
            if m == 0 and k < 4:
                wt = wr_pre[k]
            else:
                wt = wr.tile([P, P], f32, tag="wr1", name="wr1")
                dma(wt[:], r1h_d[k, :, m * P:(m + 1) * P])
            for nh in range(NH):
                nc.tensor.matmul(
                    ps[:, nh * 512:(nh + 1) * 512],
                    wt[:],
                    xk[k][:, nh * 512:(nh + 1) * 512],
                    start=(k == 0), stop=(k == KD - 1),
                )
        t = hp.tile([P, B_LOC], f32, tag="hrt", name="hrt")
        for nh in range(NH):
            sl = slice(nh * 512, (nh + 1) * 512)
            nc.scalar.activation(t[:, sl], ps[:, sl], AF.Relu,
                                 bias=br1[:, m:m + 1])
        hrt.append(t)

    # ---- heads: logits [B-part, 10] per b-chunk; accumulate over hr chunks ----
    psR = pp.tile([P, NB * 10], f32, tag="pB", name="psR")
    for bc in range(NB):
        for k in range(NHR):
            nc.tensor.matmul(
                psR[:, bc * 10:(bc + 1) * 10],
                hrt[k][:, bc * P:(bc + 1) * P],
                wcat[:, k * 10:(k + 1) * 10],
                start=(k == 0), stop=(k == NHR - 1),
            )

    # ---- per-sample router math on [128, NB] strided views ----
    LG = sm.tile([P, NB * 10], f32, tag="LG", name="LG")
    nc.vector.tensor_copy(LG[:], psR[:])
    LP = LG[:].rearrange("p (c t) -> p c t", t=10)
    RT = sm.tile([P, NB * 100], f32, tag="RT", name="RT")
    RTv = RT[:].rearrange("p (c t) -> p c t", t=100)
    nc.vector.memset(RT[:], 0.0)

    diff = sm.tile([P, NB], f32, tag="diff", name="diff")
    nc.vector.tensor_sub(diff[:], LP[:, :, 0], LP[:, :, 1])
    # add host-provided (b_dom[0]-b_dom[1]) so argmax/softmax see biased logits
    nc.vector.tensor_scalar_add(diff[:], diff[:], cdom[:, 0:1])
    mch = sm.tile([P, NB], f32, tag="mch", name="mch")
    nc.vector.tensor_single_scalar(mch[:], diff[:], 0.0, ALU.is_ge)
    absd = sm.tile([P, NB], f32, tag="absd", name="absd")
    nc.scalar.activation(absd[:], diff[:], AF.Abs)
    wsig = sm.tile([P, NB], f32, tag="wsig", name="wsig")
    nc.scalar.activation(wsig[:], absd[:], AF.Sigmoid)
    # a = w*m -> col 8 ; b = w - a -> col 9
    nc.vector.tensor_mul(RTv[:, :, 64], wsig[:], mch[:])
    nc.vector.tensor_sub(RTv[:, :, 96], wsig[:], RTv[:, :, 64])

    # one-hot argmax (first-max tie-break) for mop (cols 2:6 -> RT 0:4)
    # and ltask (cols 6:10 -> RT 4:7+)
    mx = sm.tile([P, NB], f32, tag="mx", name="mx")
    eq = [sm.tile([P, NB], f32, tag=f"eq{i}", name=f"eq{i}") for i in range(4)]
    run = sm.tile([P, NB], f32, tag="run", name="run")
    for src_base, dst_base in ((2, 0), (6, 32)):
        nc.vector.tensor_reduce(mx[:], LP[:, :, src_base:src_base + 4],
                                axis=mybir.AxisListType.X, op=ALU.max)
        for c in range(4):
            nc.vector.tensor_tensor(eq[c][:], LP[:, :, src_base + c], mx[:],
                                    op=ALU.is_ge)
        nc.vector.tensor_copy(RTv[:, :, dst_base + 0], eq[0][:])
        nc.vector.tensor_copy(run[:], eq[0][:])
        for c in range(1, 4):
            nc.vector.scalar_tensor_tensor(
                RTv[:, :, dst_base + c], eq[c][:], 1.0, run[:],
                op0=ALU.bypass, op1=ALU.subtract)
            # P_c = eq_c - run, but only where eq_c==1; since run>=eq_c pattern:
            # eq_c*(1-run) == max(eq_c - run, 0)
            nc.vector.tensor_single_scalar(RTv[:, :, dst_base + c],
                                           RTv[:, :, dst_base + c], 0.0, ALU.max)
            if c < 3:
                nc.vector.tensor_tensor(run[:], run[:], eq[c][:], op=ALU.max)

    # ---- transpose router outputs -> [rows, B_loc] (each at partition 0) ----
    rtt_op = sm.tile([N_OPS, B_LOC], f32r, tag="rtt_op", name="rtt_op")
    rtt_task = sm.tile([N_TASKS, B_LOC], f32r, tag="rtt_task", name="rtt_task")
    rtt_a = sm.tile([1, B_LOC], f32r, tag="rtt_a", name="rtt_a")
    rtt_b = sm.tile([1, B_LOC], f32r, tag="rtt_b", name="rtt_b")
    psT = pp.tile([P, B_LOC], f32, tag="pB", name="psT")
    for bc in range(NB):
        nc.tensor.transpose(psT[0:97, bc * P:(bc + 1) * P],
                            RT[:, bc * 100:bc * 100 + 97], ident[:])
    nc.vector.tensor_copy(rtt_op[:], psT[0:4, :])
    nc.vector.tensor_copy(rtt_task[:], psT[32:36, :])
    nc.vector.tensor_copy(rtt_a[:], psT[64:65, :])
    nc.vector.tensor_copy(rtt_b[:], psT[96:97, :])

    # ---- broadcast a,b to [128, B_loc] ----
    a_s = sm.tile([P, B_LOC], f32, tag="a_s", name="a_s")
    b_s = sm.tile([P, B_LOC], f32, tag="b_s", name="b_s")
    for coef, tgt in ((rtt_a, a_s), (rtt_b, b_s)):
        ps = pp.tile([P, B_LOC], f32, tag="pA", name="ps1")
        for nh in range(NH):
            nc.tensor.matmul(ps[:, nh * 512:(nh + 1) * 512], ones1[:],
                             coef[:, nh * 512:(nh + 1) * 512],
                             start=True, stop=True)
        nc.vector.tensor_copy(tgt[:], ps[:])

    # ---- expert inputs: e_in^T = x^T + emb^T @ onehot ----
    mi, li = [], []
    for emb_d, rtt, lst, tag in ((opemb_d, rtt_op, mi, "mi"),
                                 (taskemb_d, rtt_task, li, "li")):
        for dc in range(KD):
            esl = tmp.tile([N_OPS, P], f32r, tag="esl", name="esl", bufs=2)
            dma(esl[:], emb_d[:, dc * P:(dc + 1) * P])
            ps = pp.tile([P, B_LOC], f32, tag="pB", name="ps2")
            for nh in range(NH):
                nc.tensor.matmul(ps[:, nh * 512:(nh + 1) * 512],
                                 esl[:],
                                 rtt[:, nh * 512:(nh + 1) * 512],
                                 start=True, stop=True)
            t = big.tile([P, B_LOC], f32r, tag=tag)
            nc.vector.tensor_add(t[:], ps[:], xk[dc][:])
            lst.append(t)

    # ---- late consts: expert/fusion biases ----
    bm1 = sm.tile([P, HE // P], f32, tag="bm1", name="bm1")
    dma(bm1[:], bm1_d[:])
    bl1 = sm.tile([P, HE // P], f32, tag="bl1", name="bl1")
    dma(bl1[:], bl1_d[:])
    bf1 = sm.tile([P, HF // P], f32, tag="bf1", name="bf1")
    dma(bf1[:], bf1_d[:])
    bm2 = sm.tile([P, KD], f32, tag="bm2", name="bm2")
    dma(bm2[:], bm2_d[:])
    bl2 = sm.tile([P, KD], f32, tag="bl2", name="bl2")
    dma(bl2[:], bl2_d[:])
    bf2 = sm.tile([P, KD], f32, tag="bf2", name="bf2")
    dma(bf2[:], bf2_d[:])

    # ---- experts (dense both, fused+scaled accumulation into acc) ----
    acc = [big.tile([P, B_LOC], f32, tag="acc", name="acc") for _ in range(KD)]
    first_contrib = [True] * KD

    def expert(ein, w1d, w2d, b1, b2, coef):
        for q in range(NQ):
            h1 = []
            for hc in range(NHEQ):
                hcg = q * NHEQ + hc
                wt = wts.tile([P, NHEQ * P], f32r, tag="wk", name="wk")
                dma(wt[:], w1d[hcg])
                w3 = wt[:].rearrange("p (k c) -> p k c", c=P)
                ps = pp.tile([P, B_LOC], f32, tag="pA", name="ps1")
                for k in range(KD):
                    for nh in range(NH):
                        nc.tensor.matmul(ps[:, nh * 512:(nh + 1) * 512],
                                         w3[:, k, :],
                                         ein[k][:, nh * 512:(nh + 1) * 512],
                                         start=(k == 0), stop=(k == KD - 1))
                t = big.tile([P, B_LOC], f32r, tag="xin", name="h1")
                nc.scalar.activation(t[:], ps[:], AF.Relu,
                                     bias=b1[:, hcg:hcg + 1])
                h1.append(t)
            for dc in range(KD):
                wt = wts.tile([P, NHEQ * P], f32r, tag="wk", name="wk")
                dma(wt[:], w2d[q, dc])
                w3 = wt[:].rearrange("p (k c) -> p k c", c=P)
                ps = pp.tile([P, B_LOC], f32, tag="pB", name="ps2")
                for k in range(NHEQ):
                    for nh in range(NH):
                        nc.tensor.matmul(ps[:, nh * 512:(nh + 1) * 512],
                                         w3[:, k, :],
                                         h1[k][:, nh * 512:(nh + 1) * 512],
                                         start=(k == 0), stop=(k == NHEQ - 1))
                # contribution = coef * (psum [+ bias on last quarter])
                sc = b2[:, dc:dc + 1] if q == NQ - 1 else 0.0
                if first_contrib[dc]:
                    nc.vector.scalar_tensor_tensor(
                        acc[dc][:], ps[:], sc, coef[:],
                        op0=ALU.add, op1=ALU.mult)
                    first_contrib[dc] = False
                else:
                    u = tmp.tile([P, B_LOC], f32, tag="tmp", name="tmp")
                    nc.vector.scalar_tensor_tensor(
                        u[:], ps[:], sc, coef[:], op0=ALU.add, op1=ALU.mult)
                    nc.vector.tensor_add(acc[dc][:], acc[dc][:], u[:])

    expert(mi, w1h_d, w2h_d, bm1, bm2, a_s)
    expert(li, l1h_d, l2h_d, bl1, bl2, b_s)

    # ---- fused (f32r copy) + residual init of final ----
    fusedr, final = [], []
    for dc in range(KD):
        fr = big.tile([P, B_LOC], f32r, tag="mi", name="fusedr")
        nc.vector.tensor_copy(fr[:], acc[dc][:])
        fusedr.append(fr)
        fo = big.tile([P, B_LOC], f32, tag="li", name="final")
        nc.vector.tensor_scalar_add(fo[:], acc[dc][:], bf2[:, dc:dc + 1])
        final.append(fo)

    # ---- fusion MLP: final += W_f2^T @ relu(W_f1^T @ fused + b_f1) ----
    for q in range(NF):
        h1 = []
        for hc in range(NHFQ):
            hcg = q * NHFQ + hc
            wt = wts.tile([P, NHFQ * P], f32r, tag="wk", name="wk")
            dma(wt[:], f1h_d[hcg])
            w3 = wt[:].rearrange("p (k c) -> p k c", c=P)
            ps = pp.tile([P, B_LOC], f32, tag="pA", name="ps1")
            for k in range(KD):
                for nh in range(NH):
                    nc.tensor.matmul(ps[:, nh * 512:(nh + 1) * 512],
                                     w3[:, k, :],
                                     fusedr[k][:, nh * 512:(nh + 1) * 512],
                                     start=(k == 0), stop=(k == KD - 1))
            t = big.tile([P, B_LOC], f32r, tag="xin", name="h1")
            nc.scalar.activation(t[:], ps[:], AF.Relu, bias=bf1[:, hcg:hcg + 1])
            h1.append(t)
        for dc in range(KD):
            wt = wts.tile([P, NHFQ * P], f32r, tag="wk", name="wk")
            dma(wt[:], f2h_d[q, dc])
            w3 = wt[:].rearrange("p (k c) -> p k c", c=P)
            ps = pp.tile([P, B_LOC], f32, tag="pB", name="ps2")
            for k in range(NHFQ):
                for nh in range(NH):
                    nc.tensor.matmul(ps[:, nh * 512:(nh + 1) * 512],
                                     w3[:, k, :],
                                     h1[k][:, nh * 512:(nh + 1) * 512],
                                     start=(k == 0), stop=(k == NHFQ - 1))
            for nh in range(NH):
                sl = slice(nh * 512, (nh + 1) * 512)
                nc.vector.tensor_add(final[dc][:, sl], final[dc][:, sl],
                                     ps[:, sl])
                dma(outT_d[dc * P:(dc + 1) * P, sl], final[dc][:, sl])


def _build():
    from contextlib import ExitStack
    nc = bacc.Bacc("TRN2", target_bir_lowering=False, debug=False,
                   num_devices=NCORES)
    d = mybir.dt

    def din(name, shape, dt_):
        return nc.dram_tensor(name, shape, dt_, kind="ExternalInput").ap()

    xT_d = din("xT", [D, B_LOC], f32)
    r1h_d = din("r1h", [KD, P, HR], f32)
    wcat_d = din("wcat", [P, 4 * 10], f32)
    w1h_d = din("w1h", [HE // P, P, NHEQ * P], f32r)
    w2h_d = din("w2h", [NQ, KD, P, NHEQ * P], f32r)
    l1h_d = din("l1h", [HE // P, P, NHEQ * P], f32r)
    l2h_d = din("l2h", [NQ, KD, P, NHEQ * P], f32r)
    f1h_d = din("f1h", [HF // P, P, NHFQ * P], f32r)
    f2h_d = din("f2h", [NF, KD, P, NHFQ * P], f32r)
    opemb_d = din("opemb", [N_OPS, D], f32r)
    taskemb_d = din("taskemb", [N_TASKS, D], f32r)
    br1_d = din("br1", [P, NHR], f32)
    bm1_d = din("bm1", [P, HE // P], f32)
    bm2_d = din("bm2", [P, KD], f32)
    bl1_d = din("bl1", [P, HE // P], f32)
    bl2_d = din("bl2", [P, KD], f32)
    bf1_d = din("bf1", [P, HF // P], f32)
    bf2_d = din("bf2", [P, KD], f32)
    cdom_d = din("cdom", [P, 1], f32)
    outT_d = nc.dram_tensor("outT", [D, B_LOC], f32,
                            kind="ExternalOutput").ap()

    dram = (xT_d, r1h_d, wcat_d, w1h_d, w2h_d, l1h_d, l2h_d, f1h_d, f2h_d,
            opemb_d, taskemb_d, br1_d, bm1_d, bm2_d, bl1_d, bl2_d, bf1_d,
            bf2_d, cdom_d, outT_d)

    with tile.TileContext(nc) as tc:
        with ExitStack() as ctx:
            _emit(nc, tc, ctx, dram)
    nc.compile()
    return nc


_nc_cache = None


def _prep_weights(i):
    """Host-side layout packing (pure reshapes/transposes, no math)."""
    def c(a):
        return np.ascontiguousarray(a, dtype=np.float32)

    W_r1 = np.asarray(i["W_r1"], np.float32)
    r1h = c(W_r1.reshape(KD, P, HR))
    wcat = np.concatenate([np.asarray(i["W_dom"], np.float32),
                           np.asarray(i["W_mop"], np.float32),
                           np.asarray(i["W_lt"], np.float32)], axis=1)
    wcat = c(wcat.reshape(NHR, P, 10).transpose(1, 0, 2).reshape(P, 40))

    def pack1(w, nout_chunks):  # [D, HOUT] -> [HOUT/P, P(d%), KD*P]
        hob = w.shape[1] // P
        return c(w.reshape(KD, P, hob, P).transpose(2, 1, 0, 3)
                 .reshape(hob, P, KD * P))

    def pack2(w, nq):  # [HIN, D] -> [nq, KD, P(h%), (HIN/nq/P)*P]
        hin = w.shape[0]
        nk = hin // nq // P
        return c(w.reshape(nq, nk, P, KD, P).transpose(0, 3, 2, 1, 4)
                 .reshape(nq, KD, P, nk * P))

    w1h = pack1(np.asarray(i["W_m1"], np.float32), HE // P)
    l1h = pack1(np.asarray(i["W_l1"], np.float32), HE // P)
    f1h = pack1(np.asarray(i["W_f1"], np.float32), HF // P)
    w2h = pack2(np.asarray(i["W_m2"], np.float32), NQ)
    l2h = pack2(np.asarray(i["W_l2"], np.float32), NQ)
    f2h = pack2(np.asarray(i["W_f2"], np.float32), NF)

    def bias_cols(b):  # [n*P] -> [P, n]
        b = np.asarray(b, np.float32)
        return c(b.reshape(-1, P).T)

    b_dom = np.asarray(i["b_dom"], np.float32)
    cdom = np.full((P, 1), float(b_dom[0]) - float(b_dom[1]), np.float32)

    return {
        "r1h": r1h, "wcat": wcat,
        "w1h": w1h, "w2h": w2h, "l1h": l1h, "l2h": l2h,
        "f1h": f1h, "f2h": f2h,
        "opemb": c(i["op_emb"]), "taskemb": c(i["task_emb"]),
        "br1": bias_cols(i["b_r1"]), "bm1": bias_cols(i["b_m1"]),
        "bm2": bias_cols(i["b_m2"]), "bl1": bias_cols(i["b_l1"]),
        "bl2": bias_cols(i["b_l2"]), "bf1": bias_cols(i["b_f1"]),
        "bf2": bias_cols(i["b_f2"]), "cdom": cdom,
    }


def kernel(_trace=False, **inputs):
    global _nc_cache, last_exec_time_ns
    if _nc_cache is None:
        _nc_cache = _build()
    nc = _nc_cache

    shared = _prep_weights(inputs)
    x = np.asarray(inputs["x"], np.float32)
    xT = np.ascontiguousarray(x.T)  # [D, B]

    in_maps = []
    for cidx in range(NCORES):
        m = dict(shared)
        m["xT"] = np.ascontiguousarray(
            xT[:, cidx * B_LOC:(cidx + 1) * B_LOC])
        in_maps.append(m)

    res = run_bass_kernel_spmd(nc, in_maps, list(range(NCORES)),
                               trace=bool(_trace))
    last_exec_time_ns = res.exec_time_ns
    outs = [res.results[c]["outT"] for c in range(NCORES)]
    return np.ascontiguousarray(
        np.concatenate(outs, axis=1).T).astype(np.float32)

